# revision 12
# baseline (speedup 1.0000x reference)
"""Trainium2 Bass kernel for MultiHeadAttention (B=4, S=2048, D=1024, H=16, hd=64).

Sharding: 8 cores = batch(4) x head-group(2 groups of 8 heads).
Each core computes its batch's attention for its 8 heads plus the partial
output projection; the host sums the two partials per batch and adds bo.

v2 schedule (PE-bound design, ~193us of matmul streaming):
  - Warm-up LDWEIGHTS chain + dummy ACT op at t=0: HAM un-throttles and the
    exp table set loads during the initial DMA window.
  - Input DMAs split across sync (x ss0/ss1), scalar (weights) and gpsimd
    (x ss2/ss3) queues in consumption order, so the PE starts ~10us in.
  - Prologue: QK projection for pair 0 + V projection sc0-7 only.
  - Attention inner loop is software-pipelined one k-chunk ahead
    (scores(kc+1) issued before PV(kc)) so the ACT exp stream never waits
    on a freshly issued score matmul.
  - Remaining V/QK projections are cost-paced filler inside the attention
    loop; pair 3's filler is the output projection itself (q-tiles 0-2),
    using the then-idle psqk PSUM banks. Only sc12-15 remain in the tail.
  - Output partials stored as bf16 (host accumulates in fp32).
"""

import numpy as np
import ml_dtypes

import concourse.bass as bass
import concourse.tile as tile
import concourse.mybir as mybir
from concourse import bacc
from concourse.bass_utils import run_bass_kernel_spmd

BF16 = mybir.dt.bfloat16
F32 = mybir.dt.float32
AF = mybir.ActivationFunctionType
ALU = mybir.AluOpType

B, S, D, H = 4, 2048, 1024, 16
HD = D // H            # 64
HL = H // 2            # 8 local heads per core
NP = HL // 2           # 4 head pairs per core
SC = S // 128          # 16 s-chunks
DC = D // 128          # 8 d-chunks
QT = S // 512          # 4 q-tiles
NB_K = S // 128        # 16 k-chunks

_NC_CACHE = {}


def build_kernel(causal=True):
    key = ("nc", causal)
    if key in _NC_CACHE:
        return _NC_CACHE[key]
    nc = bacc.Bacc("TRN2", target_bir_lowering=False)

    # ---- DRAM I/O (per core) ----
    xT_d = nc.dram_tensor("xT", [D, S], BF16, kind="ExternalInput")
    wq_d = nc.dram_tensor("wq", [D, HL * HD], BF16, kind="ExternalInput")
    wk_d = nc.dram_tensor("wk", [D, HL * HD], BF16, kind="ExternalInput")
    wv_d = nc.dram_tensor("wv", [D, HL * HD], BF16, kind="ExternalInput")
    wo_d = nc.dram_tensor("wo", [HL * HD, D], BF16, kind="ExternalInput")
    bq_d = nc.dram_tensor("bq", [NP, 128, 1], F32, kind="ExternalInput")
    bv_d = nc.dram_tensor("bv", [1, HL * HD], BF16, kind="ExternalInput")
    padk_d = nc.dram_tensor("padk", [SC, 128, 1], F32, kind="ExternalInput")
    padq_d = nc.dram_tensor("padq", [128, 8 * QT], F32, kind="ExternalInput")
    tri_d = nc.dram_tensor("tri", [128, 128], BF16, kind="ExternalInput")
    out_d = nc.dram_tensor("out", [S, D], BF16, kind="ExternalOutput")

    with tile.TileContext(nc) as tc:
        with (
            tc.tile_pool(name="persist", bufs=1) as persist,
            tc.tile_pool(name="xpool", bufs=1) as xpool,
            tc.tile_pool(name="wpool", bufs=1) as wpool,
            tc.tile_pool(name="qk", bufs=1) as qkpool,
            tc.tile_pool(name="vals", bufs=1) as valpool,
            tc.tile_pool(name="probs", bufs=8) as probs_pool,
            tc.tile_pool(name="den", bufs=4) as den_pool,
            tc.tile_pool(name="wb", bufs=4) as wb_pool,
            tc.tile_pool(name="ost", bufs=4) as ost_pool,
            tc.tile_pool(name="dsc", bufs=4, space="DRAM") as dram_pool,
            tc.tile_pool(name="pspv", bufs=2, space="PSUM") as pspv,
            tc.tile_pool(name="psqk", bufs=2, space="PSUM") as psqk_pool,
            tc.tile_pool(name="ps2", bufs=2, space="PSUM") as ps2,
        ):
            # ---- warm-up: PE activity during the DMA window un-throttles
            # HAM before the first real matmul; dummy exp preloads the ACT
            # function table set (~2.7us otherwise paid at first score).
            warm_sb = persist.tile([128, 128], BF16, tag="warm")
            nc.vector.memset(warm_sb[:], 0.0)
            for _ in range(56):
                nc.tensor.ldweights(weights=warm_sb[:, :])
            dume_in = persist.tile([1, 16], F32, tag="dume_in")
            nc.vector.memset(dume_in[:], 0.0)
            dume_out = persist.tile([1, 16], BF16, tag="dume_out")
            nc.scalar.activation(out=dume_out[:], in_=dume_in[:], func=AF.Exp, scale=0.125)

            # ---- persistent small tiles (gpsimd queue; tiny) ----
            tri_sb = persist.tile([128, 128], BF16, tag="tri")
            nc.gpsimd.dma_start(out=tri_sb[:], in_=tri_d[:, :])
            bq_sb = persist.tile([128, NP], F32, tag="bq")
            nc.gpsimd.dma_start(out=bq_sb[:], in_=bq_d[:, :, :].rearrange("a p one -> p (a one)"))
            bv_sb = persist.tile([1, HL * HD], BF16, tag="bv")
            nc.gpsimd.dma_start(out=bv_sb[:], in_=bv_d[:, :])
            padk_sb = persist.tile([128, SC], F32, tag="padk")
            nc.gpsimd.dma_start(out=padk_sb[:], in_=padk_d[:, :, :].rearrange("c p one -> p (c one)"))
            padq_sb = persist.tile([128, 8 * QT], F32, tag="padq")
            nc.gpsimd.dma_start(out=padq_sb[:], in_=padq_d[:, :])
            ones_sb = persist.tile([1, 128], BF16, tag="ones")
            nc.vector.memset(ones_sb[:], 1.0)

            # ---- v_sb static layout (zeros + ones cols), before any dep ----
            # v_sb[sc] layout [128, HL, 128]: head j even -> [v(64) | 1 | 0(63)],
            # head j odd  -> [1 | 0(63) | v(64)].
            # Even j: v at cols [0:64], ones col 64  -> psum rows v:[0:64], den:64
            # Odd  j: ones col 0, v at cols [64:128] -> psum rows den:0, v:[64:128]
            v_sb = [valpool.tile([128, HL, 128], BF16, tag=f"v{sc}", name=f"v{sc}") for sc in range(SC)]
            for sc in range(SC):
                nc.vector.memset(v_sb[sc][:], 0.0)
                for j in range(HL):
                    onecol = 64 if j % 2 == 0 else 0
                    nc.vector.memset(v_sb[sc][:, j, onecol : onecol + 1], 1.0)

            # ---- bulk loads, split across queues in consumption order ----
            xT_sb = [xpool.tile([128, S], BF16, tag=f"xT{dc}", name=f"xT{dc}") for dc in range(DC)]
            wv_sb = [wpool.tile([128, HL * HD], BF16, tag=f"wv{dc}", name=f"wv{dc}") for dc in range(DC)]
            wq_sb = [wpool.tile([128, HL * HD], BF16, tag=f"wq{dc}", name=f"wq{dc}") for dc in range(DC)]
            wk_sb = [wpool.tile([128, HL * HD], BF16, tag=f"wk{dc}", name=f"wk{dc}") for dc in range(DC)]
            wo_sb = [wpool.tile([128, D], BF16, tag=f"wo{cc}", name=f"wo{cc}") for cc in range(4)]

            # scalar queue: pair-0 q/k weight columns + wv, then it is
            # exp-only (weight DMAs must NOT sit ahead of ACTIVATEs)
            for dc in range(DC):
                nc.scalar.dma_start(out=wq_sb[dc][:, 0:128], in_=wq_d[bass.ts(dc, 128), 0:128])
                nc.scalar.dma_start(out=wk_sb[dc][:, 0:128], in_=wk_d[bass.ts(dc, 128), 0:128])
            for dc in range(DC):
                nc.scalar.dma_start(out=wv_sb[dc][:], in_=wv_d[bass.ts(dc, 128), :])
            # sync queue: x in ss order, then remaining q/k weights, then wo
            for ss in range(4):
                for dc in range(DC):
                    nc.sync.dma_start(
                        out=xT_sb[dc][:, bass.ts(ss, 512)],
                        in_=xT_d[bass.ts(dc, 128), bass.ts(ss, 512)],
                    )
            for dc in range(DC):
                nc.sync.dma_start(out=wq_sb[dc][:, 128:512], in_=wq_d[bass.ts(dc, 128), 128:512])
                nc.sync.dma_start(out=wk_sb[dc][:, 128:512], in_=wk_d[bass.ts(dc, 128), 128:512])
            for cc in range(4):
                nc.sync.dma_start(out=wo_sb[cc][:], in_=wo_d[bass.ts(cc, 128), :])

            # ---- unit generators (for prologue + paced filler) ----
            qT_sb = [qkpool.tile([128, S], BF16, tag=f"qT{p}", name=f"qT{p}") for p in range(NP)]
            kT_sb = [qkpool.tile([128, S], BF16, tag=f"kT{p}", name=f"kT{p}") for p in range(NP)]

            def qk_units(p, ss_list):
                """(cost_us, emit_fn) units for pair p's Q/K projection."""
                units = []
                for ss in ss_list:
                    state = {}

                    def mk_q_mm(p, ss, pair, state):
                        def fn():
                            if pair == 0:
                                state["psq"] = psqk_pool.tile([128, 512], F32, tag="psqk", name="psq")
                            for dc in (2 * pair, 2 * pair + 1):
                                nc.tensor.matmul(
                                    state["psq"][:],
                                    lhsT=wq_sb[dc][:, bass.ts(p, 128)],
                                    rhs=xT_sb[dc][:, bass.ts(ss, 512)],
                                    start=(dc == 0),
                                    stop=(dc == DC - 1),
                                )
                        return fn

                    def mk_q_bias(p, ss, state):
                        def fn():
                            nc.vector.tensor_scalar_add(
                                out=qT_sb[p][:, bass.ts(ss, 512)],
                                in0=state["psq"][:],
                                scalar1=bq_sb[:, p : p + 1],
                            )
                        return fn

                    def mk_k_mm(p, ss, pair, state):
                        def fn():
                            if pair == 0:
                                state["psk"] = psqk_pool.tile([128, 512], F32, tag="psqk", name="psk")
                            for dc in (2 * pair, 2 * pair + 1):
                                nc.tensor.matmul(
                                    state["psk"][:],
                                    lhsT=wk_sb[dc][:, bass.ts(p, 128)],
                                    rhs=xT_sb[dc][:, bass.ts(ss, 512)],
                                    start=(dc == 0),
                                    stop=(dc == DC - 1),
                                )
                        return fn

                    def mk_k_copy(p, ss, state):
                        def fn():
                            nc.vector.tensor_copy(
                                out=kT_sb[p][:, bass.ts(ss, 512)],
                                in_=state["psk"][:],
                            )
                        return fn

                    for pair in range(4):
                        units.append((0.43, mk_q_mm(p, ss, pair, state)))
                    units.append((0.05, mk_q_bias(p, ss, state)))
                    for pair in range(4):
                        units.append((0.43, mk_k_mm(p, ss, pair, state)))
                    units.append((0.05, mk_k_copy(p, ss, state)))
                return units

            def emit_v(sc):
                psv = psqk_pool.tile([128, 512], F32, tag="psqk", name="psv")
                for dc in range(DC):
                    nc.tensor.matmul(
                        psv[:],
                        lhsT=xT_sb[dc][:, bass.ts(sc, 128)],
                        rhs=wv_sb[dc][:],
                        start=(dc == 0),
                        stop=False,
                    )
                nc.tensor.matmul(
                    psv[:],
                    lhsT=ones_sb[0:1, :],
                    rhs=bv_sb[0:1, :],
                    start=False,
                    stop=True,
                )
                v4 = v_sb[sc][:].rearrange("p (e two) c -> p e two c", two=2)
                ps4 = psv[:].rearrange("p (e two c) -> p e two c", two=2, c=64)
                nc.vector.tensor_scalar_mul(
                    out=v4[:, :, 0:1, 0:64],
                    in0=ps4[:, :, 0:1, :],
                    scalar1=padk_sb[:, sc : sc + 1],
                )
                nc.vector.tensor_scalar_mul(
                    out=v4[:, :, 1:2, 64:128],
                    in0=ps4[:, :, 1:2, :],
                    scalar1=padk_sb[:, sc : sc + 1],
                )

            def v_units(sc_list):
                return [(1.9, (lambda sc_: lambda: emit_v(sc_))(sc)) for sc in sc_list]

            # ---- prologue compute: QK pair0 (ss0, ss1), V sc0-3 ----
            for _, fn in qk_units(0, [0, 1]):
                fn()
            for sc in range(4):
                emit_v(sc)

            # ---- attention state ----
            valsT_sb = [valpool.tile([128, S], BF16, tag=f"valsT{cc}", name=f"valsT{cc}") for cc in range(NP)]
            den_all = persist.tile([128, NP * 32], F32, tag="den_all")

            def norm_finish(p, qts=None, fast=False):
                """Reciprocal + pad_q fold + broadcast + multiply for pair p's
                values. fast=True routes the multiplies to the vector engine
                (used for pair 3, whose normalization gates the overlapped
                output projection)."""
                for qt in range(QT) if qts is None else qts:
                    c0 = p * 32 + qt * 8
                    rcol = den_pool.tile([128, 8], F32, tag="rcol", name="rcol")
                    nc.vector.reciprocal(out=rcol[:], in_=den_all[:, c0 : c0 + 8])
                    wcol = den_pool.tile([128, 8], BF16, tag="wcol", name="wcol")
                    nc.vector.tensor_mul(
                        out=wcol[:],
                        in0=rcol[:],
                        in1=padq_sb[:, qt * 8 : (qt + 1) * 8],
                    )
                    for half in (0, 1):
                        hoff = half * 64
                        scr_b = dram_pool.tile([1, 512], BF16, tag=f"scrb{half}", name="scr_b")
                        nc.gpsimd.dma_start(
                            out=scr_b[0:1, :].rearrange("a (p f) -> (a p) f", p=128),
                            in_=wcol[:, half * 4 : (half + 1) * 4],
                        )
                        wb = wb_pool.tile([128, 512], BF16, tag="wb", name="wb")
                        nc.gpsimd.dma_start(
                            out=wb[hoff : hoff + 64, :],
                            in_=scr_b[0:1, :].to_broadcast([64, 512]),
                        )
                        mule = nc.vector if fast else nc.gpsimd
                        mule.tensor_mul(
                            out=valsT_sb[p][hoff : hoff + 64, bass.ts(qt, 512)],
                            in0=valsT_sb[p][hoff : hoff + 64, bass.ts(qt, 512)],
                            in1=wb[hoff : hoff + 64, :],
                        )

            # ---- output projection unit: one (sc, do) accumulation ----
            def emit_out(sc, do):
                pso = psqk_pool.tile([128, 512], F32, tag="psqk", name="pso")
                for cc in range(NP):
                    nc.tensor.matmul(
                        pso[:],
                        lhsT=valsT_sb[cc][:, bass.ts(sc, 128)],
                        rhs=wo_sb[cc][:, bass.ds(do * 512, 512)],
                        start=(cc == 0),
                        stop=(cc == NP - 1),
                    )
                ost = ost_pool.tile([128, 512], BF16, tag="ost")
                nc.vector.tensor_copy(out=ost[:], in_=pso[:])
                nc.sync.dma_start(
                    out=out_d[bass.ts(sc, 128), bass.ds(do * 512, 512)],
                    in_=ost[:],
                )

            def out_units(sc_list):
                us = []
                for sc in sc_list:
                    for do in range(2):
                        us.append((0.85, (lambda s, d: lambda: emit_out(s, d))(sc, do)))
                return us

            # sc12-15 depend on pair 3's last q-tile: pre-accumulate the
            # pair 0-2 contributions into SBUF during pair 3's attention so
            # the tail only adds the last pair's matmul.
            part_sb = {}

            def emit_out_partial(sc, do):
                pso = psqk_pool.tile([128, 512], F32, tag="psqk", name="psop")
                for cc in range(NP - 1):
                    nc.tensor.matmul(
                        pso[:],
                        lhsT=valsT_sb[cc][:, bass.ts(sc, 128)],
                        rhs=wo_sb[cc][:, bass.ds(do * 512, 512)],
                        start=(cc == 0),
                        stop=(cc == NP - 2),
                    )
                prt = persist.tile([128, 512], F32, tag=f"prt{sc}_{do}")
                nc.vector.tensor_copy(out=prt[:], in_=pso[:])
                part_sb[(sc, do)] = prt

            def emit_out_final(sc, do):
                pso = psqk_pool.tile([128, 512], F32, tag="psqk", name="psof")
                cc = NP - 1
                nc.tensor.matmul(
                    pso[:],
                    lhsT=valsT_sb[cc][:, bass.ts(sc, 128)],
                    rhs=wo_sb[cc][:, bass.ds(do * 512, 512)],
                    start=True,
                    stop=True,
                )
                ost = ost_pool.tile([128, 512], BF16, tag="ost")
                nc.vector.tensor_add(out=ost[:], in0=pso[:], in1=part_sb[(sc, do)][:])
                nc.sync.dma_start(
                    out=out_d[bass.ts(sc, 128), bass.ds(do * 512, 512)],
                    in_=ost[:],
                )

            def out_partial_units():
                us = []
                for sc in range(12, 16):
                    for do in range(2):
                        us.append((0.65, (lambda s, d: lambda: emit_out_partial(s, d))(sc, do)))
                return us

            # ---- per-pair filler chains; force[qt] = #units that must be
            # emitted before q-tile qt starts (scores need qT/kT of its
            # s-chunk; PV needs v_sb of its k-range).
            chain0 = (v_units([4, 5, 6, 7]) + qk_units(0, [2]) + v_units([8, 9])
                      + qk_units(0, [3]) + v_units([10, 11, 12, 13, 14, 15])
                      + qk_units(1, [0, 1, 2, 3]))
            chains = {
                0: chain0,
                1: qk_units(2, [0, 1, 2, 3]),
                2: qk_units(3, [0, 1, 2, 3]),
            }
            # chain0 prefix indices: V4-7 = 4 units, + qk0ss2 (10) = 14,
            # + V8-9 (2) + qk0ss3 (10) = 26
            forced0 = {1: 4, 2: 14, 3: 26}

            # ---- attention: software-pipelined scores/exp one kc ahead ----
            for p in range(NP):
                chain = list(chains.get(p, []))
                chain_cost = sum(c for c, _ in chain)
                spent = [0.0]
                consumed = [0]

                def consume_to(budget):
                    while chain and spent[0] < budget:
                        c, fn = chain.pop(0)
                        fn()
                        spent[0] += c
                        consumed[0] += 1

                def consume_units(n):
                    while chain and consumed[0] < n:
                        c, fn = chain.pop(0)
                        fn()
                        spent[0] += c
                        consumed[0] += 1

                n_kc_pair = sum((4 * qt + 4 if causal else 16) for qt in range(QT))
                rate = chain_cost / n_kc_pair if n_kc_pair else 0.0
                kc_global = 0

                out_chain = []  # pair 3 only: output-projection units
                out_spent = [0.0]

                def consume_out(budget):
                    while out_chain and out_spent[0] < budget:
                        c, fn = out_chain.pop(0)
                        fn()
                        out_spent[0] += c

                if p > 0:
                    # previous pair's normalization: den_all rows are already
                    # in flight; emit before this pair's attention so pair 3's
                    # overlapped out-projection never waits on it
                    norm_finish(p - 1, fast=(p == NP - 1))

                for qt in range(QT):
                    nkc = 4 * qt + 4 if causal else 16

                    if p == 0 and causal and qt in forced0:
                        consume_units(forced0[qt])
                    if p == NP - 1 and qt > 0:
                        # normalize pair 3's previous q-tile, then its output
                        # projection becomes this q-tile's filler
                        norm_finish(p, [qt - 1], fast=True)
                        out_chain.extend(out_units(range(4 * (qt - 1), 4 * qt)))
                        if qt == QT - 1:
                            out_chain.extend(out_partial_units())
                    out_rate = (out_spent[0] + sum(c for c, _ in out_chain)) / max(1, nkc)

                    def kc_geom(kc):
                        qs0 = max(qt * 512, kc * 128) if causal else qt * 512
                        return qs0, (qt + 1) * 512 - qs0

                    def emit_scores(kc):
                        qs0, width = kc_geom(kc)
                        psc2 = ps2.tile([128, 1024], F32, tag="ps2", name="psc2")
                        for half in (0, 1):
                            hoff = half * 64
                            nc.tensor.matmul(
                                psc2[:, bass.ds(half * 512, width)],
                                lhsT=kT_sb[p][hoff : hoff + 64, bass.ts(kc, 128)],
                                rhs=qT_sb[p][hoff : hoff + 64, bass.ds(qs0, width)],
                                start=True,
                                stop=True,
                            )
                        return psc2

                    def emit_exp(kc, psc2):
                        qs0, width = kc_geom(kc)
                        pt = probs_pool.tile([128, 1024], BF16, tag="probs", name="pt")
                        if width == 512:
                            nc.scalar.activation(
                                out=pt[:], in_=psc2[:], func=AF.Exp, scale=0.125
                            )
                        else:
                            nc.scalar.activation(
                                out=pt[:].rearrange("a (h w) -> a h w", h=2)[:, :, :width],
                                in_=psc2[:].rearrange("a (h w) -> a h w", h=2)[:, :, :width],
                                func=AF.Exp,
                                scale=0.125,
                            )
                        return pt

                    ppv = {}
                    for half in (0, 1):
                        ppv[half] = pspv.tile([128, 512], F32, tag="pspv", name="ppv")

                    psc = emit_scores(0)
                    pts = {0: emit_exp(0, psc)}
                    for kc in range(nkc):
                        if kc + 1 < nkc:
                            psc2 = emit_scores(kc + 1)
                            pts[kc + 1] = emit_exp(kc + 1, psc2)
                        pt = pts.pop(kc)
                        qs0, width = kc_geom(kc)
                        if causal and kc >= 4 * qt:
                            nc.vector.tensor_mul(
                                out=pt[:, 0:128], in0=pt[:, 0:128], in1=tri_sb[:]
                            )
                            nc.vector.tensor_mul(
                                out=pt[:, 512:640], in0=pt[:, 512:640], in1=tri_sb[:]
                            )
                        for half in (0, 1):
                            j = 2 * p + half
                            nc.tensor.matmul(
                                ppv[half][:, bass.ds(qs0 - qt * 512, width)],
                                lhsT=v_sb[kc][:, j, :],
                                rhs=pt[:, bass.ds(half * 512, width)],
                                start=(kc == 0),
                                stop=(kc == nkc - 1),
                            )
                        kc_global += 1
                        consume_to(kc_global * rate)
                        if p == NP - 1:
                            consume_out((kc + 1) * out_rate)

                    # ---- denominator collection + PSUM drain ----
                    sbrow = den_pool.tile([128, 512], F32, tag="sbrow", name="sbrow")
                    for half in (0, 1):
                        dr = 64 if half == 0 else 0
                        hoff = half * 64
                        nc.vector.tensor_copy(
                            out=sbrow[dr : dr + 1, :],
                            in_=ppv[half][dr : dr + 1, :],
                        )
                        nc.vector.tensor_copy(
                            out=valsT_sb[p][hoff : hoff + 64, bass.ts(qt, 512)],
                            in_=ppv[half][hoff : hoff + 64, :],
                        )
                    for half in (0, 1):
                        dr = 64 if half == 0 else 0
                        scr_a = dram_pool.tile([1, 512], F32, tag=f"scra{half}", name="scr_a")
                        nc.sync.dma_start(out=scr_a[:], in_=sbrow[dr : dr + 1, :])
                        c0 = p * 32 + qt * 8 + half * 4
                        nc.sync.dma_start(
                            out=den_all[:, c0 : c0 + 4],
                            in_=scr_a[0:1, :].rearrange(
                                "a (p f) -> (a p) f", p=128
                            ),
                        )

                # flush remaining filler so the next pair's qT/kT are ready
                consume_to(float("inf"))
                if p == NP - 1:
                    consume_out(float("inf"))

            # ---- tail: last q-tile normalization + remaining out-proj ----
            norm_finish(NP - 1, [QT - 1], fast=True)
            for sc in range(12, 16):
                for do in range(2):
                    emit_out_final(sc, do)

    nc.compile()
    _NC_CACHE[key] = nc
    return nc


def _prep_core_inputs(x, pad_mask, Wqkv, bqkv, Wo, b, hg):
    """Host-side shard prep for core (batch b, head-group hg)."""
    bf16 = ml_dtypes.bfloat16
    xT = np.ascontiguousarray(x[b].T).astype(bf16)  # [D, S]
    wq = np.empty((D, HL * HD), np.float32)
    wk = np.empty((D, HL * HD), np.float32)
    wv = np.empty((D, HL * HD), np.float32)
    bq = np.empty(HL * HD, np.float32)
    bv = np.empty(HL * HD, np.float32)
    for j in range(HL):
        gh = hg * HL + j
        r0 = gh * 3 * HD
        wq[:, j * HD : (j + 1) * HD] = Wqkv[r0 : r0 + HD, :].T
        wk[:, j * HD : (j + 1) * HD] = Wqkv[r0 + HD : r0 + 2 * HD, :].T
        wv[:, j * HD : (j + 1) * HD] = Wqkv[r0 + 2 * HD : r0 + 3 * HD, :].T
        bq[j * HD : (j + 1) * HD] = bqkv[r0 : r0 + HD]
        bv[j * HD : (j + 1) * HD] = bqkv[r0 + 2 * HD : r0 + 3 * HD]
    wo = np.ascontiguousarray(Wo[:, hg * HL * HD : (hg + 1) * HL * HD].T)  # [512, D]
    pad = pad_mask[b].astype(np.float32)  # [S]
    # padq in denominator-column layout: [pp, qt*8 + half*4 + i] =
    # pad[qt*512 + pp*4 + i], duplicated across the two halves.
    pq = pad.reshape(QT, 128, 4).transpose(1, 0, 2)  # [pp, qt, i]
    padq = np.ascontiguousarray(
        np.stack([pq, pq], axis=2).reshape(128, QT * 8)
    )
    tri = np.triu(np.ones((128, 128), np.float32))  # tri[k, q] = 1 if k <= q
    return {
        "xT": xT,
        "wq": wq.astype(bf16),
        "wk": wk.astype(bf16),
        "wv": wv.astype(bf16),
        "wo": wo.astype(bf16),
        "bq": bq.reshape(NP, 128, 1),
        "bv": bv.reshape(1, HL * HD).astype(bf16),
        "padk": pad.reshape(SC, 128, 1),
        "padq": padq,
        "tri": tri.astype(bf16),
    }


def run_sharded(inputs, trace=False):
    """Returns (full_output, BassKernelResults)."""
    x = np.asarray(inputs["x"], np.float32)
    pad_mask = np.asarray(inputs["pad_mask"])
    Wqkv = np.asarray(inputs["Wqkv"], np.float32)
    bqkv = np.asarray(inputs["bqkv"], np.float32)
    Wo = np.asarray(inputs["Wo"], np.float32)
    bo = np.asarray(inputs["bo"], np.float32)

    causal = bool(np.asarray(inputs.get("atn_mask", 1)).item())
    nc = build_kernel(causal=causal)
    in_maps = [
        _prep_core_inputs(x, pad_mask, Wqkv, bqkv, Wo, c // 2, c % 2)
        for c in range(8)
    ]
    res = run_bass_kernel_spmd(nc, in_maps, core_ids=list(range(8)), trace=trace)
    out = np.empty((B, S, D), np.float32)
    for b in range(B):
        out[b] = (res.results[2 * b]["out"].astype(np.float32)
                  + res.results[2 * b + 1]["out"].astype(np.float32) + bo)
    return out, res


def kernel(**inputs):
    out, _ = run_sharded(inputs, trace=False)
    return out


# ---------------------------------------------------------------- benchmarking
def _build_sharded_exec(nc, n_cores=8):
    """Mirror bass2jax.run_bass_via_pjrt's multi-core path, reusable for
    repeated timed executions (keeps donation semantics)."""
    import jax
    import numpy as _np
    from jax.experimental.shard_map import shard_map
    from jax.sharding import Mesh, PartitionSpec, NamedSharding
    from concourse import bass2jax as b2j
    import concourse.mybir as _mybir

    b2j.install_neuronx_cc_hook()
    partition_name = nc.partition_id_tensor.name if nc.partition_id_tensor else None
    in_names, out_names, out_avals, zero_outs = [], [], [], []
    for alloc in nc.m.functions[0].allocations:
        if not isinstance(alloc, _mybir.MemoryLocationSet):
            continue
        name = alloc.memorylocations[0].name
        if alloc.kind == "ExternalInput":
            if name != partition_name:
                in_names.append(name)
        elif alloc.kind == "ExternalOutput":
            shape = tuple(alloc.tensor_shape)
            dtype = _mybir.dt.np(alloc.dtype)
            out_names.append(name)
            out_avals.append(jax.core.ShapedArray(shape, dtype))
            zero_outs.append(_np.zeros(shape, dtype))
    n_params = len(in_names)
    in_names = in_names + out_names
    donate = tuple(range(n_params, n_params + len(out_names)))

    def _body(*args):
        operands = list(args)
        if partition_name is not None:
            operands.append(b2j.partition_id_tensor())
        outs = b2j._bass_exec_p.bind(
            *operands,
            out_avals=tuple(out_avals),
            in_names=tuple(in_names),
            out_names=tuple(out_names),
            lowering_input_output_aliases=(),
            sim_require_finite=True,
            sim_require_nnan=True,
            nc=nc,
        )
        return tuple(outs)

    if partition_name is not None:
        in_names = in_names + [partition_name]
    devices = jax.devices()[:n_cores]
    mesh = Mesh(_np.asarray(devices), ("core",))
    spec = PartitionSpec("core")
    fn = jax.jit(
        shard_map(_body, mesh=mesh, in_specs=(spec,) * (n_params + len(out_names)),
                  out_specs=(spec,) * len(out_names), check_rep=False),
        donate_argnums=donate,
        keep_unused=True,
    )
    sharding = NamedSharding(mesh, spec)
    return fn, in_names[:n_params], out_names, zero_outs, sharding


def bench(inputs, iters=6):
    """Time repeated sharded executions. Returns (per_call_s list, outputs)."""
    import jax, time
    x = np.asarray(inputs["x"], np.float32)
    pad_mask = np.asarray(inputs["pad_mask"])
    Wqkv = np.asarray(inputs["Wqkv"], np.float32)
    bqkv = np.asarray(inputs["bqkv"], np.float32)
    Wo = np.asarray(inputs["Wo"], np.float32)

    nc = build_kernel()
    in_maps = [
        _prep_core_inputs(x, pad_mask, Wqkv, bqkv, Wo, c // 2, c % 2)
        for c in range(8)
    ]
    fn, in_names, out_names, zero_outs, sharding = _build_sharded_exec(nc)
    concat_in = [
        np.concatenate([np.asarray(in_maps[c][k]) for c in range(8)], axis=0)
        for k in in_names
    ]
    dev_in = [jax.device_put(a, sharding) for a in concat_in]
    zeros_proto = [np.zeros((8 * z.shape[0], *z.shape[1:]), z.dtype) for z in zero_outs]

    times = []
    out = None
    for it in range(iters + 1):
        dz = [jax.device_put(z, sharding) for z in zeros_proto]
        jax.block_until_ready(dz)
        t0 = time.perf_counter()
        out = fn(*dev_in, *dz)
        jax.block_until_ready(out)
        t1 = time.perf_counter()
        if it > 0:  # skip compile/warmup call
            times.append(t1 - t0)
    return times, out


# revision 19
# speedup vs baseline: 1.0777x; 1.0777x over previous
"""Trainium2 Bass kernel for MultiHeadAttention (B=4, S=2048, D=1024, H=16, hd=64).

Sharding: 8 cores = batch(4) x head-group(2 groups of 8 heads).
Each core computes its batch's attention for its 8 heads plus the partial
output projection; the host sums the two partials per batch and adds bo.

v2 schedule (PE-bound design, ~193us of matmul streaming):
  - Warm-up LDWEIGHTS chain + dummy ACT op at t=0: HAM un-throttles and the
    exp table set loads during the initial DMA window.
  - Input DMAs split across sync (x ss0/ss1), scalar (weights) and gpsimd
    (x ss2/ss3) queues in consumption order, so the PE starts ~10us in.
  - Prologue: QK projection for pair 0 + V projection sc0-7 only.
  - Attention inner loop is software-pipelined one k-chunk ahead
    (scores(kc+1) issued before PV(kc)) so the ACT exp stream never waits
    on a freshly issued score matmul.
  - Remaining V/QK projections are cost-paced filler inside the attention
    loop; pair 3's filler is the output projection itself (q-tiles 0-2),
    using the then-idle psqk PSUM banks. Only sc12-15 remain in the tail.
  - Output partials stored as bf16 (host accumulates in fp32).
"""

import numpy as np
import ml_dtypes

import concourse.bass as bass
import concourse.tile as tile
import concourse.mybir as mybir
from concourse import bacc
from concourse.bass_utils import run_bass_kernel_spmd

BF16 = mybir.dt.bfloat16
F32 = mybir.dt.float32
AF = mybir.ActivationFunctionType
ALU = mybir.AluOpType

B, S, D, H = 4, 2048, 1024, 16
HD = D // H            # 64
HL = H // 2            # 8 local heads per core
NP = HL // 2           # 4 head pairs per core
SC = S // 128          # 16 s-chunks
DC = D // 128          # 8 d-chunks
QT = S // 512          # 4 q-tiles
NB_K = S // 128        # 16 k-chunks

_NC_CACHE = {}


def build_kernel(causal=True):
    key = ("nc", causal)
    if key in _NC_CACHE:
        return _NC_CACHE[key]
    nc = bacc.Bacc("TRN2", target_bir_lowering=False)

    # ---- DRAM I/O (per core) ----
    xT_d = nc.dram_tensor("xT", [D, S], BF16, kind="ExternalInput")
    wq_d = nc.dram_tensor("wq", [D, HL * HD], BF16, kind="ExternalInput")
    wk_d = nc.dram_tensor("wk", [D, HL * HD], BF16, kind="ExternalInput")
    wv_d = nc.dram_tensor("wv", [D, HL * HD], BF16, kind="ExternalInput")
    wo_d = nc.dram_tensor("wo", [HL * HD, D], BF16, kind="ExternalInput")
    bq_d = nc.dram_tensor("bq", [NP, 128, 1], F32, kind="ExternalInput")
    bv_d = nc.dram_tensor("bv", [1, HL * HD], BF16, kind="ExternalInput")
    padk_d = nc.dram_tensor("padk", [SC, 128, 1], F32, kind="ExternalInput")
    padq_d = nc.dram_tensor("padq", [128, 8 * QT], F32, kind="ExternalInput")
    tri_d = nc.dram_tensor("tri", [128, 128], BF16, kind="ExternalInput")
    out_d = nc.dram_tensor("out", [S, D], BF16, kind="ExternalOutput")

    with tile.TileContext(nc) as tc:
        with (
            tc.tile_pool(name="persist", bufs=1) as persist,
            tc.tile_pool(name="xpool", bufs=1) as xpool,
            tc.tile_pool(name="wpool", bufs=1) as wpool,
            tc.tile_pool(name="qk", bufs=1) as qkpool,
            tc.tile_pool(name="vals", bufs=1) as valpool,
            tc.tile_pool(name="probs", bufs=8) as probs_pool,
            tc.tile_pool(name="den", bufs=4) as den_pool,
            tc.tile_pool(name="wb", bufs=4) as wb_pool,
            tc.tile_pool(name="ost", bufs=4) as ost_pool,
            tc.tile_pool(name="dsc", bufs=4, space="DRAM") as dram_pool,
            tc.tile_pool(name="pspv", bufs=2, space="PSUM") as pspv,
            tc.tile_pool(name="psqk", bufs=2, space="PSUM") as psqk_pool,
            tc.tile_pool(name="ps2", bufs=2, space="PSUM") as ps2,
        ):
            # ---- warm-up: PE activity during the DMA window un-throttles
            # HAM before the first real matmul (LDWEIGHTS alone does NOT
            # count as HAM activity — measured — so use dummy matmuls);
            # dummy exp preloads the ACT function table set (~2.7us
            # otherwise paid at the first score).
            warm_sb = persist.tile([128, 128], BF16, tag="warm")
            nc.vector.memset(warm_sb[:], 0.0)
            psw = ps2.tile([128, 1024], F32, tag="ps2", name="psw")
            for _ in range(52):
                nc.tensor.matmul(
                    psw[:, 0:128], lhsT=warm_sb[:, :], rhs=warm_sb[:, :],
                    start=True, stop=True,
                )
            dume_in = persist.tile([1, 16], F32, tag="dume_in")
            nc.vector.memset(dume_in[:], 0.0)
            dume_out = persist.tile([1, 16], BF16, tag="dume_out")
            nc.scalar.activation(out=dume_out[:], in_=dume_in[:], func=AF.Exp, scale=0.125)

            # ---- persistent small tiles (gpsimd queue; tiny) ----
            tri_sb = persist.tile([128, 128], BF16, tag="tri")
            nc.gpsimd.dma_start(out=tri_sb[:], in_=tri_d[:, :])
            bq_sb = persist.tile([128, NP], F32, tag="bq")
            nc.gpsimd.dma_start(out=bq_sb[:], in_=bq_d[:, :, :].rearrange("a p one -> p (a one)"))
            bv_sb = persist.tile([1, HL * HD], BF16, tag="bv")
            nc.gpsimd.dma_start(out=bv_sb[:], in_=bv_d[:, :])
            padk_sb = persist.tile([128, SC], F32, tag="padk")
            nc.gpsimd.dma_start(out=padk_sb[:], in_=padk_d[:, :, :].rearrange("c p one -> p (c one)"))
            padq_sb = persist.tile([128, 8 * QT], F32, tag="padq")
            nc.gpsimd.dma_start(out=padq_sb[:], in_=padq_d[:, :])
            ones_sb = persist.tile([1, 128], BF16, tag="ones")
            nc.vector.memset(ones_sb[:], 1.0)

            # ---- v_sb static layout (zeros + ones cols), before any dep ----
            # v_sb[sc] layout [128, HL, 128]: head j even -> [v(64) | 1 | 0(63)],
            # head j odd  -> [1 | 0(63) | v(64)].
            # Even j: v at cols [0:64], ones col 64  -> psum rows v:[0:64], den:64
            # Odd  j: ones col 0, v at cols [64:128] -> psum rows den:0, v:[64:128]
            v_sb = [valpool.tile([128, HL, 128], BF16, tag=f"v{sc}", name=f"v{sc}") for sc in range(SC)]
            for sc in range(SC):
                nc.vector.memset(v_sb[sc][:], 0.0)
                for j in range(HL):
                    onecol = 64 if j % 2 == 0 else 0
                    nc.vector.memset(v_sb[sc][:, j, onecol : onecol + 1], 1.0)

            # ---- bulk loads, split across queues in consumption order ----
            xT_sb = [xpool.tile([128, S], BF16, tag=f"xT{dc}", name=f"xT{dc}") for dc in range(DC)]
            wv_sb = [wpool.tile([128, HL * HD], BF16, tag=f"wv{dc}", name=f"wv{dc}") for dc in range(DC)]
            wq_sb = [wpool.tile([128, HL * HD], BF16, tag=f"wq{dc}", name=f"wq{dc}") for dc in range(DC)]
            wk_sb = [wpool.tile([128, HL * HD], BF16, tag=f"wk{dc}", name=f"wk{dc}") for dc in range(DC)]
            wo_sb = [wpool.tile([128, D], BF16, tag=f"wo{cc}", name=f"wo{cc}") for cc in range(4)]

            # scalar queue: pair-0 q/k weight columns + wv, then it is
            # exp-only (weight DMAs must NOT sit ahead of ACTIVATEs)
            for dc in range(DC):
                nc.scalar.dma_start(out=wq_sb[dc][:, 0:128], in_=wq_d[bass.ts(dc, 128), 0:128])
                nc.scalar.dma_start(out=wk_sb[dc][:, 0:128], in_=wk_d[bass.ts(dc, 128), 0:128])
            for dc in range(DC):
                nc.scalar.dma_start(out=wv_sb[dc][:], in_=wv_d[bass.ts(dc, 128), :])
            # sync queue: x in need-order (ss0, ss1, then the second half as
            # one wide transfer per dc — per-DMA overhead is ~600ns on the
            # queue engine, so fewer+bigger wins), then wo
            for ss in range(2):
                for dc in range(DC):
                    nc.sync.dma_start(
                        out=xT_sb[dc][:, bass.ts(ss, 512)],
                        in_=xT_d[bass.ts(dc, 128), bass.ts(ss, 512)],
                    )
            for dc in range(DC):
                nc.sync.dma_start(
                    out=xT_sb[dc][:, 1024:2048],
                    in_=xT_d[bass.ts(dc, 128), 1024:2048],
                )
            for cc in range(4):
                nc.sync.dma_start(out=wo_sb[cc][:], in_=wo_d[bass.ts(cc, 128), :])
            # gpsimd queue: remaining q/k weight columns (needed by filler
            # from ~pair-0 mid-attention onward)
            for dc in range(DC):
                nc.gpsimd.dma_start(out=wq_sb[dc][:, 128:512], in_=wq_d[bass.ts(dc, 128), 128:512])
                nc.gpsimd.dma_start(out=wk_sb[dc][:, 128:512], in_=wk_d[bass.ts(dc, 128), 128:512])

            # ---- unit generators (for prologue + paced filler) ----
            qT_sb = [qkpool.tile([128, S], BF16, tag=f"qT{p}", name=f"qT{p}") for p in range(NP)]
            kT_sb = [qkpool.tile([128, S], BF16, tag=f"kT{p}", name=f"kT{p}") for p in range(NP)]

            def qk_units(p, ss_list):
                """(cost_us, emit_fn) units for pair p's Q/K projection."""
                units = []
                for ss in ss_list:
                    state = {}

                    def mk_q_mm(p, ss, pair, state):
                        def fn():
                            if pair == 0:
                                state["psq"] = psqk_pool.tile([128, 512], F32, tag="psqk", name="psq")
                            for dc in (2 * pair, 2 * pair + 1):
                                nc.tensor.matmul(
                                    state["psq"][:],
                                    lhsT=wq_sb[dc][:, bass.ts(p, 128)],
                                    rhs=xT_sb[dc][:, bass.ts(ss, 512)],
                                    start=(dc == 0),
                                    stop=(dc == DC - 1),
                                )
                        return fn

                    def mk_q_bias(p, ss, state):
                        def fn():
                            nc.vector.tensor_scalar_add(
                                out=qT_sb[p][:, bass.ts(ss, 512)],
                                in0=state["psq"][:],
                                scalar1=bq_sb[:, p : p + 1],
                            )
                        return fn

                    def mk_k_mm(p, ss, pair, state):
                        def fn():
                            if pair == 0:
                                state["psk"] = psqk_pool.tile([128, 512], F32, tag="psqk", name="psk")
                            for dc in (2 * pair, 2 * pair + 1):
                                nc.tensor.matmul(
                                    state["psk"][:],
                                    lhsT=wk_sb[dc][:, bass.ts(p, 128)],
                                    rhs=xT_sb[dc][:, bass.ts(ss, 512)],
                                    start=(dc == 0),
                                    stop=(dc == DC - 1),
                                )
                        return fn

                    def mk_k_copy(p, ss, state):
                        def fn():
                            nc.vector.tensor_copy(
                                out=kT_sb[p][:, bass.ts(ss, 512)],
                                in_=state["psk"][:],
                            )
                        return fn

                    for pair in range(4):
                        units.append((0.43, mk_q_mm(p, ss, pair, state)))
                    units.append((0.05, mk_q_bias(p, ss, state)))
                    for pair in range(4):
                        units.append((0.43, mk_k_mm(p, ss, pair, state)))
                    units.append((0.05, mk_k_copy(p, ss, state)))
                return units

            def emit_v(sc):
                psv = psqk_pool.tile([128, 512], F32, tag="psqk", name="psv")
                for dc in range(DC):
                    nc.tensor.matmul(
                        psv[:],
                        lhsT=xT_sb[dc][:, bass.ts(sc, 128)],
                        rhs=wv_sb[dc][:],
                        start=(dc == 0),
                        stop=False,
                    )
                nc.tensor.matmul(
                    psv[:],
                    lhsT=ones_sb[0:1, :],
                    rhs=bv_sb[0:1, :],
                    start=False,
                    stop=True,
                )
                v4 = v_sb[sc][:].rearrange("p (e two) c -> p e two c", two=2)
                ps4 = psv[:].rearrange("p (e two c) -> p e two c", two=2, c=64)
                nc.vector.tensor_scalar_mul(
                    out=v4[:, :, 0:1, 0:64],
                    in0=ps4[:, :, 0:1, :],
                    scalar1=padk_sb[:, sc : sc + 1],
                )
                nc.vector.tensor_scalar_mul(
                    out=v4[:, :, 1:2, 64:128],
                    in0=ps4[:, :, 1:2, :],
                    scalar1=padk_sb[:, sc : sc + 1],
                )

            def v_units(sc_list):
                return [(1.9, (lambda sc_: lambda: emit_v(sc_))(sc)) for sc in sc_list]

            # ---- prologue compute: QK pair0 ss0, V sc0-3 ----
            for _, fn in qk_units(0, [0]):
                fn()
            for sc in range(4):
                emit_v(sc)

            # ---- attention state ----
            valsT_sb = [valpool.tile([128, S], BF16, tag=f"valsT{cc}", name=f"valsT{cc}") for cc in range(NP)]
            den_all = persist.tile([128, NP * 32], F32, tag="den_all")

            def norm_finish(p, qts=None, fast=False):
                """Reciprocal + pad_q fold + broadcast + multiply for pair p's
                values. fast=True routes the multiplies to the vector engine
                (used for pair 3, whose normalization gates the overlapped
                output projection)."""
                for qt in range(QT) if qts is None else qts:
                    c0 = p * 32 + qt * 8
                    rcol = den_pool.tile([128, 8], F32, tag="rcol", name="rcol")
                    nc.vector.reciprocal(out=rcol[:], in_=den_all[:, c0 : c0 + 8])
                    wcol = den_pool.tile([128, 8], BF16, tag="wcol", name="wcol")
                    nc.vector.tensor_mul(
                        out=wcol[:],
                        in0=rcol[:],
                        in1=padq_sb[:, qt * 8 : (qt + 1) * 8],
                    )
                    for half in (0, 1):
                        hoff = half * 64
                        scr_b = dram_pool.tile([1, 512], BF16, tag=f"scrb{half}", name="scr_b")
                        nc.gpsimd.dma_start(
                            out=scr_b[0:1, :].rearrange("a (p f) -> (a p) f", p=128),
                            in_=wcol[:, half * 4 : (half + 1) * 4],
                        )
                        wb = wb_pool.tile([128, 512], BF16, tag="wb", name="wb")
                        nc.gpsimd.dma_start(
                            out=wb[hoff : hoff + 64, :],
                            in_=scr_b[0:1, :].to_broadcast([64, 512]),
                        )
                        mule = nc.vector if fast else nc.gpsimd
                        mule.tensor_mul(
                            out=valsT_sb[p][hoff : hoff + 64, bass.ts(qt, 512)],
                            in0=valsT_sb[p][hoff : hoff + 64, bass.ts(qt, 512)],
                            in1=wb[hoff : hoff + 64, :],
                        )

            # ---- output projection unit: one (sc, do) accumulation ----
            def emit_out(sc, do):
                pso = psqk_pool.tile([128, 512], F32, tag="psqk", name="pso")
                for cc in range(NP):
                    nc.tensor.matmul(
                        pso[:],
                        lhsT=valsT_sb[cc][:, bass.ts(sc, 128)],
                        rhs=wo_sb[cc][:, bass.ds(do * 512, 512)],
                        start=(cc == 0),
                        stop=(cc == NP - 1),
                    )
                ost = ost_pool.tile([128, 512], BF16, tag="ost")
                nc.vector.tensor_copy(out=ost[:], in_=pso[:])
                nc.sync.dma_start(
                    out=out_d[bass.ts(sc, 128), bass.ds(do * 512, 512)],
                    in_=ost[:],
                )

            def out_units(sc_list):
                us = []
                for sc in sc_list:
                    for do in range(2):
                        us.append((0.85, (lambda s, d: lambda: emit_out(s, d))(sc, do)))
                return us

            # sc12-15 depend on pair 3's last q-tile: pre-accumulate the
            # pair 0-2 contributions into SBUF during pair 3's attention so
            # the tail only adds the last pair's matmul.
            part_sb = {}

            def emit_out_partial(sc, do):
                pso = psqk_pool.tile([128, 512], F32, tag="psqk", name="psop")
                for cc in range(NP - 1):
                    nc.tensor.matmul(
                        pso[:],
                        lhsT=valsT_sb[cc][:, bass.ts(sc, 128)],
                        rhs=wo_sb[cc][:, bass.ds(do * 512, 512)],
                        start=(cc == 0),
                        stop=(cc == NP - 2),
                    )
                prt = persist.tile([128, 512], F32, tag=f"prt{sc}_{do}")
                nc.vector.tensor_copy(out=prt[:], in_=pso[:])
                part_sb[(sc, do)] = prt

            def emit_out_final(sc, do):
                pso = psqk_pool.tile([128, 512], F32, tag="psqk", name="psof")
                cc = NP - 1
                nc.tensor.matmul(
                    pso[:],
                    lhsT=valsT_sb[cc][:, bass.ts(sc, 128)],
                    rhs=wo_sb[cc][:, bass.ds(do * 512, 512)],
                    start=True,
                    stop=True,
                )
                ost = ost_pool.tile([128, 512], BF16, tag="ost")
                nc.vector.tensor_add(out=ost[:], in0=pso[:], in1=part_sb[(sc, do)][:])
                nc.sync.dma_start(
                    out=out_d[bass.ts(sc, 128), bass.ds(do * 512, 512)],
                    in_=ost[:],
                )

            def out_partial_units():
                us = []
                for sc in range(12, 16):
                    for do in range(2):
                        us.append((0.65, (lambda s, d: lambda: emit_out_partial(s, d))(sc, do)))
                return us

            # ---- per-pair filler chains; force[qt] = #units that must be
            # emitted before q-tile qt starts (scores need qT/kT of its
            # s-chunk; PV needs v_sb of its k-range).
            chain0 = (qk_units(0, [1]) + v_units([4, 5, 6, 7]) + qk_units(0, [2])
                      + v_units([8, 9]) + qk_units(0, [3])
                      + v_units([10, 11, 12, 13, 14, 15])
                      + qk_units(1, [0, 1, 2, 3]))
            chains = {
                0: chain0,
                1: qk_units(2, [0, 1, 2, 3]),
                2: qk_units(3, [0, 1, 2, 3]),
            }
            # chain0 prefix: qk0ss1 (10) + V4-7 (4) = 14 before qt1,
            # + qk0ss2 (10) = 24 before qt2, + V8-9 + qk0ss3 = 36 before qt3
            forced0 = {1: 14, 2: 24, 3: 36}

            # ---- attention: software-pipelined scores/exp one kc ahead ----
            for p in range(NP):
                chain = list(chains.get(p, []))
                chain_cost = sum(c for c, _ in chain)
                spent = [0.0]
                consumed = [0]

                def consume_to(budget):
                    while chain and spent[0] < budget:
                        c, fn = chain.pop(0)
                        fn()
                        spent[0] += c
                        consumed[0] += 1

                def consume_units(n):
                    while chain and consumed[0] < n:
                        c, fn = chain.pop(0)
                        fn()
                        spent[0] += c
                        consumed[0] += 1

                n_kc_pair = sum((4 * qt + 4 if causal else 16) for qt in range(QT))
                rate = chain_cost / n_kc_pair if n_kc_pair else 0.0
                kc_global = 0

                out_chain = []  # pair 3 only: output-projection units
                out_spent = [0.0]

                def consume_out(budget):
                    while out_chain and out_spent[0] < budget:
                        c, fn = out_chain.pop(0)
                        fn()
                        out_spent[0] += c

                for qt in range(QT):
                    nkc = 4 * qt + 4 if causal else 16

                    if p == 0 and causal and qt in forced0:
                        consume_units(forced0[qt])

                    def kc_geom(kc):
                        qs0 = max(qt * 512, kc * 128) if causal else qt * 512
                        return qs0, (qt + 1) * 512 - qs0

                    def emit_scores(kc):
                        qs0, width = kc_geom(kc)
                        psc2 = ps2.tile([128, 1024], F32, tag="ps2", name="psc2")
                        for half in (0, 1):
                            hoff = half * 64
                            nc.tensor.matmul(
                                psc2[:, bass.ds(half * 512, width)],
                                lhsT=kT_sb[p][hoff : hoff + 64, bass.ts(kc, 128)],
                                rhs=qT_sb[p][hoff : hoff + 64, bass.ds(qs0, width)],
                                start=True,
                                stop=True,
                            )
                        return psc2

                    def emit_exp(kc, psc2):
                        qs0, width = kc_geom(kc)
                        pt = probs_pool.tile([128, 1024], BF16, tag="probs", name="pt")
                        if width == 512:
                            nc.scalar.activation(
                                out=pt[:], in_=psc2[:], func=AF.Exp, scale=0.125
                            )
                        else:
                            nc.scalar.activation(
                                out=pt[:].rearrange("a (h w) -> a h w", h=2)[:, :, :width],
                                in_=psc2[:].rearrange("a (h w) -> a h w", h=2)[:, :, :width],
                                func=AF.Exp,
                                scale=0.125,
                            )
                        return pt

                    ppv = {}
                    for half in (0, 1):
                        ppv[half] = pspv.tile([128, 512], F32, tag="pspv", name="ppv")

                    psc = emit_scores(0)
                    pts = {0: emit_exp(0, psc)}
                    for kc in range(nkc):
                        if kc + 1 < nkc:
                            psc2 = emit_scores(kc + 1)
                            pts[kc + 1] = emit_exp(kc + 1, psc2)
                        pt = pts.pop(kc)
                        qs0, width = kc_geom(kc)
                        if causal and kc >= 4 * qt:
                            nc.vector.tensor_mul(
                                out=pt[:, 0:128], in0=pt[:, 0:128], in1=tri_sb[:]
                            )
                            nc.vector.tensor_mul(
                                out=pt[:, 512:640], in0=pt[:, 512:640], in1=tri_sb[:]
                            )
                        for half in (0, 1):
                            j = 2 * p + half
                            nc.tensor.matmul(
                                ppv[half][:, bass.ds(qs0 - qt * 512, width)],
                                lhsT=v_sb[kc][:, j, :],
                                rhs=pt[:, bass.ds(half * 512, width)],
                                start=(kc == 0),
                                stop=(kc == nkc - 1),
                            )
                        if p == NP - 1 and qt > 0 and kc == 3:
                            # normalize pair 3's previous q-tile (the den DMA
                            # from qt-1's drain has landed by now, so the
                            # reciprocal doesn't head-of-line-block the DVE);
                            # its output projection becomes paced filler
                            norm_finish(p, [qt - 1], fast=True)
                            out_chain.extend(out_units(range(4 * (qt - 1), 4 * qt)))
                            if qt == 2:
                                out_chain.extend(out_partial_units())
                        kc_global += 1
                        consume_to(kc_global * rate)
                        if p == NP - 1 and kc >= 6:
                            rem_kc = max(1, n_kc_pair - kc_global - 2)
                            out_rate = sum(c for c, _ in out_chain) / rem_kc
                            consume_out(out_spent[0] + out_rate)

                    # ---- denominator collection + PSUM drain ----
                    sbrow = den_pool.tile([128, 512], F32, tag="sbrow", name="sbrow")
                    for half in (0, 1):
                        dr = 64 if half == 0 else 0
                        hoff = half * 64
                        nc.vector.tensor_copy(
                            out=sbrow[dr : dr + 1, :],
                            in_=ppv[half][dr : dr + 1, :],
                        )
                        nc.vector.tensor_copy(
                            out=valsT_sb[p][hoff : hoff + 64, bass.ts(qt, 512)],
                            in_=ppv[half][hoff : hoff + 64, :],
                        )
                    for half in (0, 1):
                        dr = 64 if half == 0 else 0
                        scr_a = dram_pool.tile([1, 512], F32, tag=f"scra{half}", name="scr_a")
                        nc.sync.dma_start(out=scr_a[:], in_=sbrow[dr : dr + 1, :])
                        c0 = p * 32 + qt * 8 + half * 4
                        nc.sync.dma_start(
                            out=den_all[:, c0 : c0 + 4],
                            in_=scr_a[0:1, :].rearrange(
                                "a (p f) -> (a p) f", p=128
                            ),
                        )

                    if qt == 0 and p > 0:
                        norm_finish(p - 1)

                # flush remaining filler so the next pair's qT/kT are ready
                consume_to(float("inf"))
                if p == NP - 1:
                    consume_out(float("inf"))

            # ---- tail: last q-tile normalization + remaining out-proj ----
            norm_finish(NP - 1, [QT - 1], fast=True)
            for sc in range(12, 16):
                for do in range(2):
                    emit_out_final(sc, do)

    nc.compile()
    _NC_CACHE[key] = nc
    return nc


def _prep_core_inputs(x, pad_mask, Wqkv, bqkv, Wo, b, hg):
    """Host-side shard prep for core (batch b, head-group hg)."""
    bf16 = ml_dtypes.bfloat16
    xT = np.ascontiguousarray(x[b].T).astype(bf16)  # [D, S]
    wq = np.empty((D, HL * HD), np.float32)
    wk = np.empty((D, HL * HD), np.float32)
    wv = np.empty((D, HL * HD), np.float32)
    bq = np.empty(HL * HD, np.float32)
    bv = np.empty(HL * HD, np.float32)
    for j in range(HL):
        gh = hg * HL + j
        r0 = gh * 3 * HD
        wq[:, j * HD : (j + 1) * HD] = Wqkv[r0 : r0 + HD, :].T
        wk[:, j * HD : (j + 1) * HD] = Wqkv[r0 + HD : r0 + 2 * HD, :].T
        wv[:, j * HD : (j + 1) * HD] = Wqkv[r0 + 2 * HD : r0 + 3 * HD, :].T
        bq[j * HD : (j + 1) * HD] = bqkv[r0 : r0 + HD]
        bv[j * HD : (j + 1) * HD] = bqkv[r0 + 2 * HD : r0 + 3 * HD]
    wo = np.ascontiguousarray(Wo[:, hg * HL * HD : (hg + 1) * HL * HD].T)  # [512, D]
    pad = pad_mask[b].astype(np.float32)  # [S]
    # padq in denominator-column layout: [pp, qt*8 + half*4 + i] =
    # pad[qt*512 + pp*4 + i], duplicated across the two halves.
    pq = pad.reshape(QT, 128, 4).transpose(1, 0, 2)  # [pp, qt, i]
    padq = np.ascontiguousarray(
        np.stack([pq, pq], axis=2).reshape(128, QT * 8)
    )
    tri = np.triu(np.ones((128, 128), np.float32))  # tri[k, q] = 1 if k <= q
    return {
        "xT": xT,
        "wq": wq.astype(bf16),
        "wk": wk.astype(bf16),
        "wv": wv.astype(bf16),
        "wo": wo.astype(bf16),
        "bq": bq.reshape(NP, 128, 1),
        "bv": bv.reshape(1, HL * HD).astype(bf16),
        "padk": pad.reshape(SC, 128, 1),
        "padq": padq,
        "tri": tri.astype(bf16),
    }


def run_sharded(inputs, trace=False):
    """Returns (full_output, BassKernelResults)."""
    x = np.asarray(inputs["x"], np.float32)
    pad_mask = np.asarray(inputs["pad_mask"])
    Wqkv = np.asarray(inputs["Wqkv"], np.float32)
    bqkv = np.asarray(inputs["bqkv"], np.float32)
    Wo = np.asarray(inputs["Wo"], np.float32)
    bo = np.asarray(inputs["bo"], np.float32)

    causal = bool(np.asarray(inputs.get("atn_mask", 1)).item())
    nc = build_kernel(causal=causal)
    in_maps = [
        _prep_core_inputs(x, pad_mask, Wqkv, bqkv, Wo, c // 2, c % 2)
        for c in range(8)
    ]
    res = run_bass_kernel_spmd(nc, in_maps, core_ids=list(range(8)), trace=trace)
    out = np.empty((B, S, D), np.float32)
    for b in range(B):
        out[b] = (res.results[2 * b]["out"].astype(np.float32)
                  + res.results[2 * b + 1]["out"].astype(np.float32) + bo)
    return out, res


def kernel(**inputs):
    out, _ = run_sharded(inputs, trace=False)
    return out


# ---------------------------------------------------------------- benchmarking
def _build_sharded_exec(nc, n_cores=8):
    """Mirror bass2jax.run_bass_via_pjrt's multi-core path, reusable for
    repeated timed executions (keeps donation semantics)."""
    import jax
    import numpy as _np
    from jax.experimental.shard_map import shard_map
    from jax.sharding import Mesh, PartitionSpec, NamedSharding
    from concourse import bass2jax as b2j
    import concourse.mybir as _mybir

    b2j.install_neuronx_cc_hook()
    partition_name = nc.partition_id_tensor.name if nc.partition_id_tensor else None
    in_names, out_names, out_avals, zero_outs = [], [], [], []
    for alloc in nc.m.functions[0].allocations:
        if not isinstance(alloc, _mybir.MemoryLocationSet):
            continue
        name = alloc.memorylocations[0].name
        if alloc.kind == "ExternalInput":
            if name != partition_name:
                in_names.append(name)
        elif alloc.kind == "ExternalOutput":
            shape = tuple(alloc.tensor_shape)
            dtype = _mybir.dt.np(alloc.dtype)
            out_names.append(name)
            out_avals.append(jax.core.ShapedArray(shape, dtype))
            zero_outs.append(_np.zeros(shape, dtype))
    n_params = len(in_names)
    in_names = in_names + out_names
    donate = tuple(range(n_params, n_params + len(out_names)))

    def _body(*args):
        operands = list(args)
        if partition_name is not None:
            operands.append(b2j.partition_id_tensor())
        outs = b2j._bass_exec_p.bind(
            *operands,
            out_avals=tuple(out_avals),
            in_names=tuple(in_names),
            out_names=tuple(out_names),
            lowering_input_output_aliases=(),
            sim_require_finite=True,
            sim_require_nnan=True,
            nc=nc,
        )
        return tuple(outs)

    if partition_name is not None:
        in_names = in_names + [partition_name]
    devices = jax.devices()[:n_cores]
    mesh = Mesh(_np.asarray(devices), ("core",))
    spec = PartitionSpec("core")
    fn = jax.jit(
        shard_map(_body, mesh=mesh, in_specs=(spec,) * (n_params + len(out_names)),
                  out_specs=(spec,) * len(out_names), check_rep=False),
        donate_argnums=donate,
        keep_unused=True,
    )
    sharding = NamedSharding(mesh, spec)
    return fn, in_names[:n_params], out_names, zero_outs, sharding


def bench(inputs, iters=6):
    """Time repeated sharded executions. Returns (per_call_s list, outputs)."""
    import jax, time
    x = np.asarray(inputs["x"], np.float32)
    pad_mask = np.asarray(inputs["pad_mask"])
    Wqkv = np.asarray(inputs["Wqkv"], np.float32)
    bqkv = np.asarray(inputs["bqkv"], np.float32)
    Wo = np.asarray(inputs["Wo"], np.float32)

    nc = build_kernel()
    in_maps = [
        _prep_core_inputs(x, pad_mask, Wqkv, bqkv, Wo, c // 2, c % 2)
        for c in range(8)
    ]
    fn, in_names, out_names, zero_outs, sharding = _build_sharded_exec(nc)
    concat_in = [
        np.concatenate([np.asarray(in_maps[c][k]) for c in range(8)], axis=0)
        for k in in_names
    ]
    dev_in = [jax.device_put(a, sharding) for a in concat_in]
    zeros_proto = [np.zeros((8 * z.shape[0], *z.shape[1:]), z.dtype) for z in zero_outs]

    times = []
    out = None
    for it in range(iters + 1):
        dz = [jax.device_put(z, sharding) for z in zeros_proto]
        jax.block_until_ready(dz)
        t0 = time.perf_counter()
        out = fn(*dev_in, *dz)
        jax.block_until_ready(out)
        t1 = time.perf_counter()
        if it > 0:  # skip compile/warmup call
            times.append(t1 - t0)
    return times, out
